# revision 1
# baseline (speedup 1.0000x reference)
"""Trainium2 Bass kernel: segment_sum of edge features into nodes (GNN aggregation).

out[n, :] = sum of edges[e, :] over edges with receivers[e] == n, for
n in [0, 100000), edges [1000000, 64] fp32 — distributed over 8 NeuronCores.
Cores are value-sharded by receiver range (12500 nodes each, disjoint), so no
cross-core reduction is needed; the host concatenates the shards.

Device algorithm (degree-slotted static-ones matmul, fused):
  - Host sorts each core's edges by receiver and packs them into "node-rows"
    of 4/2/1 slots (three regions by degree remainder, minimizing padding);
    a chunk = 128 slots.
  - The stationary operand is a STATIC block-ones matrix (e.g. [128, 32] with
    ones[s, j] = 1 iff s//4 == j): out row j = sum of row j's slots. No
    per-chunk weight generation at all (no one-hot; VectorEngine is idle).
  - Edge fp32 values ride as fp16 hi + fp16 lo halves: the matmul output
    access pattern wraps both 64-column halves onto the same PSUM columns and
    PSUM's per-element has_written accumulate folds hi+lo in hardware
    (end-to-end error ~2e-7 relative).
  - One matmul processes up to 4 chunks (rhs free dim 512, 4D wrapped out
    AP), cutting the PE instruction stream ~4x; column tiling
    (tile_position=(0, 32b)) packs 4 chunk-columns per 128-partition PSUM
    block, 8 blocks fill one 2KB PSUM bank exactly; one ScalarEngine copy
    flushes the bank. Input DMAs ride the Sync-engine HWDGE ring, output DMAs
    the Scalar-engine ring so the streams don't serialize on one FIFO.
  - Small S1/S2 groups are interleaved into the S4 stream to prime the DMA
    pipeline and pad bubbles; group sizes ramp down at the tail.
  - Host folds the ~3 rows per node with np.add.reduceat (S4) and vectorized
    adds (S2/S1), then fixes up any capacity-spilled edges.
"""

import os

import numpy as np

N_EDGES = 1_000_000
N_NODES = 100_000
N_FEAT = 64
N_CORES = 8
NODES_PER_CORE = N_NODES // N_CORES  # 12500

# group sizes (in 128-row blocks); shared by host packing and device schedule
G4 = [8] * 29 + [4, 2, 1, 1]  # 240 blocks: rows cap 30720 (mean ~30500)
G2 = [8] * 3  # 24 blocks: cap 3072 (mean ~3125; small spills possible)
G1 = [8] * 3  # 24 blocks: cap 3072
N4_BLOCKS = sum(G4)
N2_BLOCKS = sum(G2)
N1_BLOCKS = sum(G1)
N_BLOCKS = N4_BLOCKS + N2_BLOCKS + N1_BLOCKS  # 288
R4_CAP = N4_BLOCKS * 128
R2_CAP = N2_BLOCKS * 128
R1_CAP = N1_BLOCKS * 128
C4 = N4_BLOCKS * 4
C2 = N2_BLOCKS * 2
C1 = N1_BLOCKS * 1
C_CHUNKS = C4 + C2 + C1  # 1032

_NC_CACHE = None
LAST_RESULT = None


def _region_layout(groups, cpb):
    """Per-group (block_start, chunk_start) offsets within a region."""
    bs, cs, out = 0, 0, []
    for gw in groups:
        out.append((bs, cs))
        bs += gw
        cs += gw * cpb
    return out


_L4 = _region_layout(G4, 4)
_L2 = _region_layout(G2, 2)
_L1 = _region_layout(G1, 1)


def _row_maps():
    """Vectorized row-id -> (chunk index, column j) maps per region.

    Within a group of gw blocks, chunks are laid out b-major (lc = b*gw +
    blkin) so one matmul's rhs spans up to 4 consecutive chunks of the same
    column-group b across blocks.
    """
    maps = {}
    for name, groups, layout, cpb, cbase in (
        ("s4", G4, _L4, 4, 0),
        ("s2", G2, _L2, 2, C4),
        ("s1", G1, _L1, 1, C4 + C2),
    ):
        cap = sum(groups) * 128
        rows = np.arange(cap)
        block = rows >> 7
        gstarts = np.array([b for b, _ in layout])
        g = np.searchsorted(gstarts, block, side="right") - 1
        blkin = block - gstarts[g]
        gw = np.array(groups)[g]
        cstart = cbase + np.array([c for _, c in layout])[g]
        rows_per_col = 128 // cpb  # 32 / 64 / 128
        b = (rows & 127) // rows_per_col
        j = (rows & 127) % rows_per_col
        lc = cstart + b * gw + blkin
        maps[name] = (lc.astype(np.int64), j.astype(np.int64))
    return maps


_ROW_MAPS = _row_maps()


def _build_nc():
    global _NC_CACHE
    if _NC_CACHE is not None:
        return _NC_CACHE

    import concourse.bass as bass
    import concourse.tile as tile
    from concourse import bacc, mybir

    F16 = mybir.dt.float16
    F32 = mybir.dt.float32

    nc = bacc.Bacc("TRN2", target_bir_lowering=False)
    tokens = nc.dram_tensor("tokens", [128, C_CHUNKS, 128], F16, kind="ExternalInput")
    ones4 = nc.dram_tensor("ones4", [128, 32], F16, kind="ExternalInput")
    ones2 = nc.dram_tensor("ones2", [128, 64], F16, kind="ExternalInput")
    ones1 = nc.dram_tensor("ones1", [128, 128], F16, kind="ExternalInput")
    out = nc.dram_tensor("out", [128, N_BLOCKS, 64], F32, kind="ExternalOutput")

    with tile.TileContext(nc) as tc:
        with (
            tc.tile_pool(name="const", bufs=1) as const,
            tc.tile_pool(name="tok", bufs=6) as tokp,
            tc.tile_pool(name="ps", bufs=4, space="PSUM") as psp,
            tc.tile_pool(name="stage", bufs=3) as stp,
        ):
            ones4_t = const.tile([128, 32], F16)
            nc.scalar.dma_start(ones4_t[:], ones4[:])
            ones2_t = const.tile([128, 64], F16)
            nc.scalar.dma_start(ones2_t[:], ones2[:])
            ones1_t = const.tile([128, 128], F16)
            nc.scalar.dma_start(ones1_t[:], ones1[:])

            # (groups, layout, cpb, ones tile, out-rows/chunk, cbase, bbase)
            regs = [
                (G1, _L1, 1, ones1_t, 128, C4 + C2, N4_BLOCKS + N2_BLOCKS),
                (G2, _L2, 2, ones2_t, 64, C4, N4_BLOCKS),
                (G4, _L4, 4, ones4_t, 32, 0, 0),
            ]
            emit = [(0, 0), (1, 0)]
            small = [(0, k) for k in range(1, len(G1))] + [
                (1, k) for k in range(1, len(G2))
            ]
            for k in range(len(G4)):
                emit.append((2, k))
                if k % 3 == 2 and small:
                    emit.append(small.pop(0))
            emit += small

            flush_tick = 0
            for ridx, gidx in emit:
                groups, layout, cpb, ones_t, cols, cbase, bbase = regs[ridx]
                gw = groups[gidx]
                b0, cs = layout[gidx]
                c0 = cbase + cs
                nchunks = gw * cpb
                tok = tokp.tile([128, 8 * 4, 128], F16, tag="tok")
                nc.sync.dma_start(
                    tok[:, 0:nchunks, :], tokens[:, c0 : c0 + nchunks, :])
                ps = psp.tile([128, 8 * 64], F32, tag="ps")
                for b in range(cpb):
                    for blk0 in range(0, gw, 4):
                        w = min(4, gw - blk0)
                        pslice = ps[cols * b : cols * (b + 1),
                                    blk0 * 64 : (blk0 + w) * 64]
                        o = bass.AP(
                            pslice.tensor, pslice.offset,
                            [list(pslice.ap[0]), [64, w], [0, 2], [1, 64]])
                        nc.tensor.matmul(
                            out=o, lhsT=ones_t[:],
                            rhs=tok[:, b * gw + blk0 : b * gw + blk0 + w, :],
                            start=True, stop=True,
                            tile_position=(0, cols * b))
                stage = stp.tile([128, 8 * 64], F32, tag="stage")
                if flush_tick % 2:
                    nc.vector.tensor_copy(stage[:, 0 : gw * 64], ps[:, 0 : gw * 64])
                else:
                    nc.scalar.copy(stage[:, 0 : gw * 64], ps[:, 0 : gw * 64])
                flush_tick += 1
                nc.scalar.dma_start(
                    out[:, bbase + b0 : bbase + b0 + gw, :],
                    stage[:, 0 : gw * 64])
    nc.compile()
    _NC_CACHE = nc
    return nc


def _numpy_segment_sum(edges, receivers):
    out = np.zeros((N_NODES, N_FEAT), np.float32)
    r = np.asarray(receivers).astype(np.int64)
    ok = (r >= 0) & (r < N_NODES)
    np.add.at(out, r[ok], np.asarray(edges, np.float32)[ok])
    return out


def kernel(edges, nodes, receivers):
    global LAST_RESULT

    edges = np.ascontiguousarray(edges, dtype=np.float32)
    n_nodes = nodes.shape[0]
    r = np.asarray(receivers).astype(np.int64)
    if (
        edges.shape != (N_EDGES, N_FEAT)
        or n_nodes != N_NODES
        or r.shape != (N_EDGES,)
        or os.environ.get("KERNEL_FORCE_NUMPY")
    ):
        return _numpy_segment_sum(edges, receivers)

    order = np.argsort(r, kind="stable")
    r_s = r[order]
    bounds = np.searchsorted(r_s, NODES_PER_CORE * np.arange(N_CORES + 1))

    hi_all = edges.astype(np.float16)
    lo_all = (edges - hi_all.astype(np.float32)).astype(np.float16)

    ar = np.arange(128)
    ones4 = np.zeros((128, 32), np.float16)
    ones4[ar, ar // 4] = 1.0
    ones2 = np.zeros((128, 64), np.float16)
    ones2[ar, ar // 2] = 1.0
    ones1 = np.zeros((128, 128), np.float16)
    ones1[ar, ar] = 1.0

    lc4_map, j4_map = _ROW_MAPS["s4"]
    lc2_map, j2_map = _ROW_MAPS["s2"]
    lc1_map, j1_map = _ROW_MAPS["s1"]

    in_maps = []
    spill_ids = []
    meta = []
    for i in range(N_CORES):
        lo_b, hi_b = bounds[i], bounds[i + 1]
        idx = order[lo_b:hi_b]
        rr = r_s[lo_b:hi_b] - NODES_PER_CORE * i

        d = np.bincount(rr, minlength=NODES_PER_CORE)
        rem = d & 3
        rows4_n = (d >> 2) + (rem == 3)
        rows2_n = (rem == 2).astype(np.int64)
        rows1_n = (rem == 1).astype(np.int64)
        if (
            rows4_n.sum() > R4_CAP
            or rows2_n.sum() > R2_CAP
            or rows1_n.sum() > R1_CAP
        ):
            # Spill whole tail nodes to a host-side fixup.
            cut = min(
                int(np.searchsorted(np.cumsum(rows4_n), R4_CAP, side="right")),
                int(np.searchsorted(np.cumsum(rows2_n), R2_CAP, side="right")),
                int(np.searchsorted(np.cumsum(rows1_n), R1_CAP, side="right")),
            )
            sp = rr >= cut
            spill_ids.append(idx[sp])
            idx, rr = idx[~sp], rr[~sp]
            d = np.bincount(rr, minlength=NODES_PER_CORE)
            rem = d & 3
            rows4_n = (d >> 2) + (rem == 3)
            rows2_n = (rem == 2).astype(np.int64)
            rows1_n = (rem == 1).astype(np.int64)

        def excl_cumsum(a):
            s = np.zeros_like(a)
            np.cumsum(a[:-1], out=s[1:])
            return s

        rs4 = excl_cumsum(rows4_n)
        rs2 = excl_cumsum(rows2_n)
        rs1 = excl_cumsum(rows1_n)
        node_first = excl_cumsum(d)

        rank = np.arange(len(rr)) - node_first[rr]
        e_rem = rem[rr]
        n_s4_edges = np.where(e_rem == 3, d[rr], (d[rr] >> 2) << 2)
        m4 = rank < n_s4_edges
        m2 = (~m4) & (e_rem == 2)
        m1 = (~m4) & (e_rem == 1)

        tokens = np.zeros((128, C_CHUNKS, 128), np.float16)

        row4 = rs4[rr[m4]] + (rank[m4] >> 2)
        lc = lc4_map[row4]
        p = j4_map[row4] * 4 + (rank[m4] & 3)
        tokens[p, lc, 0:64] = hi_all[idx[m4]]
        tokens[p, lc, 64:128] = lo_all[idx[m4]]

        row2 = rs2[rr[m2]]
        slot2 = rank[m2] - n_s4_edges[m2]
        lc = lc2_map[row2]
        p = j2_map[row2] * 2 + slot2
        tokens[p, lc, 0:64] = hi_all[idx[m2]]
        tokens[p, lc, 64:128] = lo_all[idx[m2]]

        row1 = rs1[rr[m1]]
        lc = lc1_map[row1]
        p = j1_map[row1]
        tokens[p, lc, 0:64] = hi_all[idx[m1]]
        tokens[p, lc, 64:128] = lo_all[idx[m1]]

        in_maps.append(
            {"tokens": tokens, "ones4": ones4, "ones2": ones2, "ones1": ones1}
        )
        meta.append((rows4_n, rs4, rows2_n, rs2, rows1_n, rs1))

    from concourse.bass_utils import run_bass_kernel_spmd

    nc = _build_nc()
    res = run_bass_kernel_spmd(nc, in_maps, core_ids=list(range(N_CORES)))
    LAST_RESULT = res

    # ---- unshard: decode device rows back to row-major order, fold per node.
    # The ROW->(block, partition) placement is unchanged (row & 127 spans the
    # block's partitions in order); only chunk order within a group differs,
    # which the out tensor never sees.
    full = np.zeros((N_NODES, N_FEAT), np.float32)
    for i in range(N_CORES):
        dev = res.results[i]["out"]  # [128, N_BLOCKS, 64]
        rows4_n, rs4, rows2_n, rs2, rows1_n, rs1 = meta[i]
        block = full[i * NODES_PER_CORE : (i + 1) * NODES_PER_CORE]

        arr4 = dev[:, 0:N4_BLOCKS, :].transpose(1, 0, 2).reshape(R4_CAP, 64)
        nz = rows4_n > 0
        if nz.any():
            block[nz] = np.add.reduceat(arr4, rs4[nz])

        arr2 = (
            dev[:, N4_BLOCKS : N4_BLOCKS + N2_BLOCKS, :]
            .transpose(1, 0, 2)
            .reshape(R2_CAP, 64)
        )
        m2n = rows2_n > 0
        if m2n.any():
            block[m2n] += arr2[rs2[m2n]]

        arr1 = (
            dev[:, N4_BLOCKS + N2_BLOCKS :, :].transpose(1, 0, 2).reshape(R1_CAP, 64)
        )
        m1n = rows1_n > 0
        if m1n.any():
            block[m1n] += arr1[rs1[m1n]]

    if spill_ids:
        sp = np.concatenate(spill_ids)
        np.add.at(full, r[sp], edges[sp])

    return full



# revision 4
# speedup vs baseline: 2.1953x; 2.1953x over previous
"""Trainium2 Bass kernel: segment_sum of edge features into nodes (GNN
aggregation).

out[n, :] = sum of edges[e, :] over edges with receivers[e] == n, for
n in [0, 100000), edges [1000000, 64] fp32 — distributed over 8 NeuronCores.
Cores are value-sharded by receiver range (12500 nodes each, disjoint), so no
cross-core reduction is needed; the host concatenates the shards.

Device algorithm ("identity-matmul tower fold", fp16 end to end):
  - Edge features ride as plain fp16 (end-to-end error ~6e-4 relative vs the
    2e-2 gate), halving input traffic vs an fp32-exact hi+lo split.
  - Host splits nodes with degree > 16 into pseudo-nodes of <= 16 edges,
    sorts pseudo-nodes by degree (desc), and packs 128 per block: pseudo-node
    j of block b puts its k-th edge row at tokens[j, c0[b] + k, :].  A block
    occupies K_b = max-degree-in-block consecutive chunks ("towers"); padding
    is only (K_b - deg) zero slots per node — a few % total, since degrees
    within a sorted block are nearly equal.
  - ONE matmul per block folds the whole tower: lhsT = identity (fp16), rhs =
    tok[:, c0:c0+K, :] (free dim K*64 <= 1024 for 16-bit), and the out access
    pattern [[part 128], [0, K], [1, 64]] revisits the same 64 PSUM columns
    for every chunk — PSUM's per-element has_written accumulate sums the K
    chunks in hardware.  PE makes a single pass over the data; VectorEngine
    stays idle; no per-chunk weights.
  - 8 blocks fill one 2KB PSUM bank exactly; one ScalarE/VectorE copy
    (alternating) casts the bank to fp16 in SBUF; outputs ride the
    Scalar-engine HWDGE ring while inputs stream on the Sync ring.  Output is
    exactly one 64-col row per pseudo-node (~1.7 MB/core vs 9.4 MB for the
    padded-row scheme).
  - Host adds pseudo-node rows back into node rows (np.add.at over ~13k rows)
    in f32.
  - Block heights K_b are measured from the actual data (elementwise max
    across the 8 cores' sorted degree profiles) and baked into the compiled
    program inside kernel(); all cores share one SPMD schedule.
"""

import os

import numpy as np

N_EDGES = 1_000_000
N_NODES = 100_000
N_FEAT = 64
N_CORES = 8
NPC = N_NODES // N_CORES  # 12500 nodes per core
K_CAP = 8  # max tower height: matmul out free iteration is capped at 512 elems
BLK = 128  # pseudo-nodes per block (one partition each)
GRP = 8  # blocks per PSUM bank == blocks per input DMA group

_NC_CACHE = {}
LAST_RESULT = None


def _excl_cumsum(a):
    s = np.zeros_like(a)
    np.cumsum(a[:-1], out=s[1:])
    return s


def _build_nc(k_sched):
    """Compile the SPMD program for a static tuple of block heights."""
    if k_sched in _NC_CACHE:
        return _NC_CACHE[k_sched]

    import concourse.bass as bass
    import concourse.tile as tile
    from concourse import bacc, mybir

    F16 = mybir.dt.float16
    F32 = mybir.dt.float32

    nb = len(k_sched)
    c0 = np.concatenate([[0], np.cumsum(k_sched)]).astype(np.int64)
    c_total = int(c0[-1])
    ngroups = (nb + GRP - 1) // GRP
    maxg = max(
        int(c0[min(nb, (g + 1) * GRP)] - c0[g * GRP]) for g in range(ngroups)
    )

    nc = bacc.Bacc("TRN2", target_bir_lowering=False)
    tokens = nc.dram_tensor("tokens", [128, c_total, 64], F16, kind="ExternalInput")
    eye = nc.dram_tensor("eye", [128, 128], F16, kind="ExternalInput")
    out = nc.dram_tensor("out", [128, nb, 64], F16, kind="ExternalOutput")

    with tile.TileContext(nc) as tc:
        with (
            tc.tile_pool(name="const", bufs=1) as const,
            tc.tile_pool(name="tok", bufs=4) as tokp,
            tc.tile_pool(name="ps", bufs=4, space="PSUM") as psp,
            tc.tile_pool(name="stage", bufs=3) as stp,
        ):
            eye_t = const.tile([128, 128], F16)
            nc.scalar.dma_start(eye_t[:], eye[:])

            for g in range(ngroups):
                b0 = g * GRP
                b1 = min(nb, b0 + GRP)
                nblk = b1 - b0
                gc0 = int(c0[b0])
                gcn = int(c0[b1]) - gc0
                tok = tokp.tile([128, maxg, 64], F16, tag="tok")
                nc.sync.dma_start(tok[:, 0:gcn, :], tokens[:, gc0 : gc0 + gcn, :])
                ps = psp.tile([128, 512], F32, tag="ps")
                for s in range(nblk):
                    k = k_sched[b0 + s]
                    if k == 0:
                        continue
                    cs = int(c0[b0 + s]) - gc0
                    pslice = ps[:, s * 64 : (s + 1) * 64]
                    o = bass.AP(
                        pslice.tensor,
                        pslice.offset,
                        [list(pslice.ap[0]), [0, k], [1, 64]],
                    )
                    nc.tensor.matmul(
                        out=o,
                        lhsT=eye_t[:],
                        rhs=tok[:, cs : cs + k, :],
                        start=True,
                        stop=True,
                    )
                stage = stp.tile([128, 512], F16, tag="stage")
                if g % 2:
                    nc.vector.tensor_copy(stage[:, 0 : nblk * 64], ps[:, 0 : nblk * 64])
                else:
                    nc.scalar.copy(stage[:, 0 : nblk * 64], ps[:, 0 : nblk * 64])
                nc.scalar.dma_start(out[:, b0:b1, :], stage[:, 0 : nblk * 64])
    nc.compile()
    _NC_CACHE[k_sched] = nc
    return nc


def _numpy_segment_sum(edges, receivers, n_nodes):
    out = np.zeros((n_nodes, edges.shape[1]), np.float32)
    r = np.asarray(receivers).astype(np.int64)
    ok = (r >= 0) & (r < n_nodes)
    np.add.at(out, r[ok], np.asarray(edges, np.float32)[ok])
    return out


def kernel(edges, nodes, receivers):
    global LAST_RESULT

    edges = np.ascontiguousarray(edges, dtype=np.float32)
    n_nodes = nodes.shape[0]
    r = np.asarray(receivers).astype(np.int64)
    if (
        edges.shape != (N_EDGES, N_FEAT)
        or n_nodes != N_NODES
        or r.shape != (N_EDGES,)
        or ((r < 0) | (r >= N_NODES)).any()
        or os.environ.get("KERNEL_FORCE_NUMPY")
    ):
        return _numpy_segment_sum(edges, receivers, n_nodes)

    order = np.argsort(r, kind="stable")
    r_s = r[order]
    bounds = np.searchsorted(r_s, NPC * np.arange(N_CORES + 1))
    hi_all = edges.astype(np.float16)

    # ---- pass 1: per-core pseudo-node construction + sorted degree profiles
    per_core = []
    nb_max = 0
    for i in range(N_CORES):
        lo_b, hi_b = bounds[i], bounds[i + 1]
        idx = order[lo_b:hi_b]
        rr = (r_s[lo_b:hi_b] - NPC * i).astype(np.int64)
        d = np.bincount(rr, minlength=NPC)
        n_parts = np.maximum((d + K_CAP - 1) // K_CAP, 1)
        pseudo_base = _excl_cumsum(n_parts)
        n_pseudo = int(n_parts.sum())
        pseudo_orig = np.repeat(np.arange(NPC), n_parts)
        part_idx = np.arange(n_pseudo) - pseudo_base[pseudo_orig]
        pseudo_deg = np.minimum(d[pseudo_orig] - K_CAP * part_idx, K_CAP)
        sort_ord = np.argsort(-pseudo_deg, kind="stable")
        inv = np.empty(n_pseudo, np.int64)
        inv[sort_ord] = np.arange(n_pseudo)
        deg_sorted = pseudo_deg[sort_ord]
        per_core.append(
            (idx, rr, d, pseudo_base, inv, pseudo_orig, sort_ord, n_pseudo, deg_sorted)
        )
        nb_max = max(nb_max, (n_pseudo + BLK - 1) // BLK)

    # Static schedule: per-block height = max over cores of block max degree.
    k_all = np.zeros((N_CORES, nb_max), np.int64)
    for i in range(N_CORES):
        deg_sorted = per_core[i][8]
        nb_i = (len(deg_sorted) + BLK - 1) // BLK
        k_all[i, :nb_i] = deg_sorted[0 : nb_i * BLK : BLK]
    k_sched_arr = k_all.max(axis=0)
    nb = int(np.max(np.nonzero(k_sched_arr)[0])) + 1 if k_sched_arr.any() else 0
    if nb == 0:
        return np.zeros((N_NODES, N_FEAT), np.float32)
    k_sched = tuple(int(x) for x in k_sched_arr[:nb])
    c0 = np.concatenate([[0], np.cumsum(k_sched)]).astype(np.int64)
    c_total = int(c0[-1])

    nc = _build_nc(k_sched)

    # ---- pass 2: scatter edges into per-core token arrays
    ar = np.arange(128)
    eye_np = np.zeros((128, 128), np.float16)
    eye_np[ar, ar] = 1.0
    in_maps = []
    for i in range(N_CORES):
        idx, rr, d, pseudo_base, inv, _, _, _, _ = per_core[i]
        node_first = _excl_cumsum(d)
        rank = np.arange(len(rr)) - node_first[rr]
        pn = pseudo_base[rr] + rank // K_CAP
        rk = rank % K_CAP
        q = inv[pn]
        blk = q >> 7
        j = q & 127
        chunk = c0[blk] + rk
        tokens = np.zeros((128, c_total, 64), np.float16)
        tokens[j, chunk, :] = hi_all[idx]
        in_maps.append({"tokens": tokens, "eye": eye_np})

    from concourse.bass_utils import run_bass_kernel_spmd

    res = run_bass_kernel_spmd(nc, in_maps, core_ids=list(range(N_CORES)))
    LAST_RESULT = res

    # ---- unshard: row q of dev out is pseudo-node sort_ord[q]'s sum.
    full = np.zeros((N_NODES, N_FEAT), np.float32)
    for i in range(N_CORES):
        dev = res.results[i]["out"]  # [128, nb, 64] f16
        rows = dev.transpose(1, 0, 2).reshape(-1, 64).astype(np.float32)
        _, _, _, _, _, pseudo_orig, sort_ord, n_pseudo, _ = per_core[i]
        m = min(n_pseudo, nb * BLK)  # trailing deg-0 pseudo-nodes may be trimmed
        block = full[i * NPC : (i + 1) * NPC]
        np.add.at(block, pseudo_orig[sort_ord[:m]], rows[:m])

    return full
